# revision 1
# baseline (speedup 1.0000x reference)
"""Trainium2 Bass kernel: 2-layer bidirectional GRU decoder + dense/softmax head.

Data-parallel over 8 NeuronCores (batch 4096 -> 512 per core). Inside each
core everything runs transposed: partition dim = hidden units with
[fwd(64); bwd(64)] packed to 128 partitions, batch on the free dim.
"""

import os
import sys

sys.path.insert(0, "/opt/trn_rl_repo")

from contextlib import ExitStack

import numpy as np

import concourse.bass as bass
import concourse.bacc as bacc
import concourse.tile as tile
from concourse import mybir
from concourse.bass_utils import run_bass_kernel_spmd

AF = mybir.ActivationFunctionType
OP = mybir.AluOpType
DT = mybir.dt

B, T, F, H, DENSE, P = 4096, 72, 9, 64, 128, 24
NCORES = 8
BC = B // NCORES  # 512 batch per core
G3 = 3 * H

# ---- tuning knobs ----
N_CHUNK = 256        # batch columns per chain (512 = 1 chain, 256 = 2 chains)
STREAM_DT = "bf16"    # "f32" (fp32 storage, fp32r matmuls) or "bf16"
MM_EXACT = False     # True: plain fp32 matmuls (4 cyc/row) for max accuracy


def _np_dt():
    return np.float32 if STREAM_DT == "f32" else np.dtype("bfloat16")


def _mb_dt():
    return DT.float32 if STREAM_DT == "f32" else DT.bfloat16


def _mm(ap):
    """Cast an AP to the matmul dtype (fp32r trick for fp32 streams)."""
    if STREAM_DT == "f32" and not MM_EXACT:
        return ap.bitcast(DT.float32r)
    return ap


WEIGHT_NAMES = (
    ["l1x_z", "l1x_r", "l1x_h"]
    + ["l1u_z", "l1u_r", "l1u_h"]
    + ["l2a_z", "l2a_r", "l2a_h"]
    + ["l2b_z", "l2b_r", "l2b_h"]
    + ["l2u_z", "l2u_r", "l2u_h"]
    + ["ident", "dense_w", "out_w"]
)
VEC_NAMES = ["br1h", "br2h", "bi2h", "bz2", "br2", "dense_b", "out_b",
             "ones_a", "ones_b"]


def build_module(split_l2_sigmoid: bool, reps: int = 1):
    nc = bacc.Bacc("TRN2", target_bir_lowering=False, debug=False)
    sdt = _mb_dt()

    # ---- DRAM I/O ----
    d_xin = nc.dram_tensor("xin", [2 * F + 1, T * BC], sdt, kind="ExternalInput").ap()
    d_h0 = nc.dram_tensor("h0", [2 * H, BC], sdt, kind="ExternalInput").ap()
    d_w = {}
    for n in WEIGHT_NAMES:
        shape = {
            "l1x_z": [2 * F + 1, 2 * H], "l1x_r": [2 * F + 1, 2 * H],
            "l1x_h": [2 * F + 1, 2 * H],
            "dense_w": [2 * H, DENSE], "out_w": [DENSE, P],
        }.get(n, [2 * H, 2 * H])
        d_w[n] = nc.dram_tensor(n, shape, sdt, kind="ExternalInput").ap()
    d_v = {}
    for n in VEC_NAMES:
        shape = {"out_b": [P, 1], "ones_a": [P, 1], "ones_b": [1, P]}.get(n, [2 * H, 1])
        d_v[n] = nc.dram_tensor(n, shape, DT.float32, kind="ExternalInput").ap()
    d_out = nc.dram_tensor("out", [P, BC], DT.float32, kind="ExternalOutput").ap()

    N = N_CHUNK
    NCH = BC // N  # number of chains

    with tile.TileContext(nc) as tc, ExitStack() as ctx:
        wpool = ctx.enter_context(tc.tile_pool(name="weights", bufs=1))
        seq_pool = ctx.enter_context(tc.tile_pool(name="seq", bufs=1))
        spool = ctx.enter_context(tc.tile_pool(name="sig", bufs=3 * NCH))
        epool = ctx.enter_context(tc.tile_pool(name="ew", bufs=3 * NCH))
        hpool = ctx.enter_context(tc.tile_pool(name="h2", bufs=2 * NCH))
        fpool = ctx.enter_context(tc.tile_pool(name="feat", bufs=1))
        opool = ctx.enter_context(tc.tile_pool(name="outs", bufs=1))
        zr_ps = ctx.enter_context(tc.tile_pool(name="zr", bufs=2 * NCH, space="PSUM"))
        nb_xh = 2 if NCH == 1 else NCH
        xh_ps = ctx.enter_context(tc.tile_pool(name="xh", bufs=nb_xh, space="PSUM"))
        rh_ps = ctx.enter_context(tc.tile_pool(name="rh", bufs=nb_xh, space="PSUM"))

        # ---- load weights ----
        w_sb = {}
        for n in WEIGHT_NAMES:
            wt = wpool.tile(list(d_w[n].shape), sdt, tag=f"w_{n}")
            nc.sync.dma_start(wt[:], d_w[n])
            w_sb[n] = wt
        v_sb = {}
        for n in VEC_NAMES:
            vt = wpool.tile(list(d_v[n].shape), DT.float32, tag=f"v_{n}")
            nc.sync.dma_start(vt[:], d_v[n])
            v_sb[n] = vt

        h0_t = wpool.tile([2 * H, BC], sdt, tag="h0t")
        nc.sync.dma_start(h0_t[:], d_h0)
        xin_sb = wpool.tile([2 * F + 1, T * BC], sdt, tag="xin_sb")
        nc.sync.dma_start(xin_sb[:], d_xin)
        zeros_t = wpool.tile([2 * H, BC], sdt, tag="zeros")
        nc.vector.memset(zeros_t[:], 0.0)

        # layer-1 output sequence, one tile per chain so the chains share no
        # tile and stay schedulable independently. Column block s holds
        # [h_fwd(time s); h_bwd(time T-1-s)] for that chain's batch columns.
        seq_t = [seq_pool.tile([2 * H, T * N], sdt, tag=f"seq{c}",
                               name=f"seq{c}")
                 for c in range(NCH)]

        def seq_sl(s, c):
            return seq_t[c][:, s * N: (s + 1) * N]

        feat = fpool.tile([2 * H, BC], sdt, tag="feat")

        rep_ctx = tc.For_i(0, reps, 1) if reps > 1 else None
        if rep_ctx is not None:
            rep_ctx.__enter__()

        def emit_mms(specs):
            """Emit matmuls assigning start/stop per PSUM bank (2KB zero
            region): first matmul into a bank starts the group, last stops."""
            banks = {}
            for i, (out_ap, lhsT, rhs) in enumerate(specs):
                bk = (id(out_ap.tensor), out_ap.offset // 512)
                banks.setdefault(bk, []).append(i)
            for i, (out_ap, lhsT, rhs) in enumerate(specs):
                bk = (id(out_ap.tensor), out_ap.offset // 512)
                nc.tensor.matmul(out_ap, lhsT, rhs,
                                 start=(banks[bk][0] == i),
                                 stop=(banks[bk][-1] == i))

        def gru_step(layer, s, c, x_specs_f, h_prev, h_out):
            """Emit one fused fwd+bwd GRU step for chain c."""
            zr = zr_ps.tile([2 * H, 2 * N], DT.float32, tag="zr")
            xh = xh_ps.tile([2 * H, N], DT.float32, tag="xh")
            rh = rh_ps.tile([2 * H, N], DT.float32, tag="rh")
            u = {g: w_sb[f"l{layer}u_{g}"] for g in "zrh"}
            zr_specs = (x_specs_f("z", zr[:, 0:N])
                        + x_specs_f("r", zr[:, N: 2 * N])
                        + [(zr[:, 0:N], _mm(u["z"][:]), _mm(h_prev)),
                           (zr[:, N: 2 * N], _mm(u["r"][:]), _mm(h_prev))])
            emit_mms(zr_specs)
            xh_specs = x_specs_f("h", xh[:, 0:N])
            for i, (out_ap, lhsT, rhs) in enumerate(xh_specs):
                # group stays open; the inject matmul closes it
                nc.tensor.matmul(out_ap, lhsT, rhs, start=(i == 0), stop=False)
            nc.tensor.matmul(rh[:], _mm(u["h"][:]), _mm(h_prev),
                             start=True, stop=True)
            # gates
            sg = spool.tile([2 * H, 2 * N], sdt, tag="sg")
            if layer == 2 and split_l2_sigmoid:
                nc.scalar.activation(sg[:, 0:N], zr[:, 0:N], AF.Sigmoid,
                                     bias=v_sb["bz2"][:])
                nc.scalar.activation(sg[:, N: 2 * N], zr[:, N: 2 * N], AF.Sigmoid,
                                     bias=v_sb["br2"][:])
            else:
                nc.scalar.activation(sg[:], zr[:], AF.Sigmoid)
            # t = (rh + br_h) * r   (reads PSUM once)
            t = epool.tile([2 * H, N], sdt, tag="t")
            brh = v_sb["br1h" if layer == 1 else "br2h"]
            nc.vector.scalar_tensor_tensor(t[:], rh[:], brh[:],
                                           sg[:, N: 2 * N], OP.add, OP.mult)
            # inject t into the xh accumulation: xh += I @ t (closes group)
            nc.tensor.matmul(xh[:], _mm(w_sb["ident"][:]), _mm(t[:]),
                             start=False, stop=True)
            # hh = act(xh + r*(rh+br_h) + bi_h)
            hh = epool.tile([2 * H, N], sdt, tag="hh")
            if layer == 1:
                nc.scalar.activation(hh[:], xh[:], AF.Relu)
            else:
                nc.scalar.activation(hh[:], xh[:], AF.Tanh,
                                     bias=v_sb["bi2h"][:])
            # h_new = hh + z*(h_prev - hh); w/zw on GPSIMD (SBUF-only ops)
            w_ = epool.tile([2 * H, N], sdt, tag="w_")
            nc.gpsimd.tensor_sub(w_[:], h_prev, hh[:])
            zw = epool.tile([2 * H, N], sdt, tag="zw")
            nc.gpsimd.tensor_mul(zw[:], sg[:, 0:N], w_[:])
            nc.vector.tensor_add(h_out, hh[:], zw[:])

        # ---- layer 1 ----
        for s in range(T):
            for c in range(NCH):
                xt = xin_sb[:, s * BC + c * N: s * BC + c * N + N]

                def l1_x(g, out_ps, _xt=xt):
                    return [(out_ps, _mm(w_sb[f"l1x_{g}"][:]), _mm(_xt))]

                h_prev = (h0_t[:, c * N: c * N + N] if s == 0
                          else seq_sl(s - 1, c))
                gru_step(1, s, c, l1_x, h_prev, seq_sl(s, c))

        # ---- layer 2 ----
        h2_prev = {c: zeros_t[:, c * N: c * N + N] for c in range(NCH)}
        for s in range(T):
            for c in range(NCH):
                x1 = seq_sl(s, c)
                x2 = seq_sl(T - 1 - s, c)

                def l2_x(g, out_ps, _x1=x1, _x2=x2):
                    return [(out_ps, _mm(w_sb[f"l2a_{g}"][:]), _mm(_x1)),
                            (out_ps, _mm(w_sb[f"l2b_{g}"][:]), _mm(_x2))]

                if s == T - 1:
                    h_out = feat[:, c * N: c * N + N]
                else:
                    h2t = hpool.tile([2 * H, N], sdt, tag="h2t")
                    h_out = h2t[:]
                gru_step(2, s, c, l2_x, h2_prev[c], h_out)
                h2_prev[c] = h_out

        # ---- head: relu(feat @ dense_W + b) -> softmax(out_W + b) ----
        ps_d = zr_ps.tile([2 * H, 2 * N], DT.float32, tag="zr")
        nc.tensor.matmul(ps_d[:, 0:BC], _mm(w_sb["dense_w"][:]), _mm(feat[:]),
                         start=True, stop=True)
        h3 = opool.tile([DENSE, BC], sdt, tag="h3")
        nc.scalar.activation(h3[:], ps_d[:, 0:BC], AF.Relu, bias=v_sb["dense_b"][:])

        ps_l = zr_ps.tile([2 * H, 2 * N], DT.float32, tag="zr", name="ps_l")
        nc.tensor.matmul(ps_l[0:P, 0:BC], _mm(w_sb["out_w"][:]), _mm(h3[:]),
                         start=True, stop=True)
        ex = opool.tile([P, BC], DT.float32, tag="ex")
        nc.scalar.activation(ex[:], ps_l[0:P, 0:BC], AF.Exp, bias=v_sb["out_b"][:])

        ones_a = opool.tile([P, 1], DT.float32, tag="ones_a2")
        nc.vector.memset(ones_a[:], 1.0)
        ps_s = zr_ps.tile([2 * H, 2 * N], DT.float32, tag="zr")
        nc.tensor.matmul(ps_s[0:1, 0:BC], ones_a[:], ex[:],
                         start=True, stop=True)
        rin = opool.tile([1, BC], DT.float32, tag="rin")
        nc.vector.reciprocal(rin[:], ps_s[0:1, 0:BC])
        ones_b = opool.tile([1, P], DT.float32, tag="ones_b2")
        nc.vector.memset(ones_b[:], 1.0)
        ps_b = zr_ps.tile([2 * H, 2 * N], DT.float32, tag="zr", name="ps_b")
        nc.tensor.matmul(ps_b[0:P, 0:BC], ones_b[:], rin[:],
                         start=True, stop=True)
        res = opool.tile([P, BC], DT.float32, tag="res")
        nc.vector.tensor_mul(res[:], ex[:], ps_b[0:P, 0:BC])
        nc.sync.dma_start(d_out, res[:])

        if rep_ctx is not None:
            rep_ctx.__exit__(None, None, None)

    nc.finalize()
    return nc


def _gs(g):
    i = "zrh".index(g)
    return slice(i * H, (i + 1) * H)


def prepare_maps(kw):
    """Host-side prep: build per-core input maps (numpy only)."""
    f32 = np.float32
    npdt = _np_dt()
    x = np.asarray(kw["x"], f32)

    wm = {}
    for g in "zrh":
        gs = _gs(g)
        l1x = np.zeros((2 * F + 1, 2 * H), f32)
        l1x[0:F, 0:H] = kw["d1f_W"][:, gs]
        l1x[F:2 * F, H:2 * H] = kw["d1b_W"][:, gs]
        bias_f = kw["d1f_bi"][gs] + (kw["d1f_br"][gs] if g != "h" else 0.0)
        bias_b = kw["d1b_bi"][gs] + (kw["d1b_br"][gs] if g != "h" else 0.0)
        l1x[2 * F, 0:H] = bias_f
        l1x[2 * F, H:2 * H] = bias_b
        wm[f"l1x_{g}"] = l1x

        for lu, uf, ub in ((f"l1u_{g}", kw["d1f_U"], kw["d1b_U"]),
                           (f"l2u_{g}", kw["d2f_U"], kw["d2b_U"])):
            m = np.zeros((2 * H, 2 * H), f32)
            m[0:H, 0:H] = uf[:, gs]
            m[H:2 * H, H:2 * H] = ub[:, gs]
            wm[lu] = m

        a = np.zeros((2 * H, 2 * H), f32)
        a[0:H, 0:H] = kw["d2f_W"][0:H, gs]
        a[H:2 * H, H:2 * H] = kw["d2b_W"][H:2 * H, gs]
        wm[f"l2a_{g}"] = a
        b_ = np.zeros((2 * H, 2 * H), f32)
        b_[0:H, H:2 * H] = kw["d2b_W"][0:H, gs]
        b_[H:2 * H, 0:H] = kw["d2f_W"][H:2 * H, gs]
        wm[f"l2b_{g}"] = b_

    wm["ident"] = np.eye(2 * H, dtype=f32)
    wm["dense_w"] = np.asarray(kw["dense_W"], f32)
    wm["out_w"] = np.asarray(kw["out_W"], f32)

    vm = {
        "br1h": np.concatenate([kw["d1f_br"][_gs("h")], kw["d1b_br"][_gs("h")]]),
        "br2h": np.concatenate([kw["d2f_br"][_gs("h")], kw["d2b_br"][_gs("h")]]),
        "bi2h": np.concatenate([kw["d2f_bi"][_gs("h")], kw["d2b_bi"][_gs("h")]]),
        "bz2": np.concatenate([kw["d2f_bi"][_gs("z")] + kw["d2f_br"][_gs("z")],
                               kw["d2b_bi"][_gs("z")] + kw["d2b_br"][_gs("z")]]),
        "br2": np.concatenate([kw["d2f_bi"][_gs("r")] + kw["d2f_br"][_gs("r")],
                               kw["d2b_bi"][_gs("r")] + kw["d2b_br"][_gs("r")]]),
        "dense_b": np.asarray(kw["dense_b"], f32),
        "out_b": np.asarray(kw["out_b"], f32),
        "ones_a": np.ones(P, f32),
        "ones_b": np.ones(P, f32),
    }
    split_l2 = bool(np.any(vm["bz2"]) or np.any(vm["br2"]))

    base = {n: np.ascontiguousarray(w.astype(npdt)) for n, w in wm.items()}
    for n, v in vm.items():
        shape = (1, P) if n == "ones_b" else (P, 1) if n in ("out_b", "ones_a") \
            else (2 * H, 1)
        base[n] = np.ascontiguousarray(np.asarray(v, f32).reshape(shape))

    in_maps = []
    for c in range(NCORES):
        bs = slice(c * BC, (c + 1) * BC)
        xc = x[bs]  # (BC, T, F)
        xin = np.empty((2 * F + 1, T, BC), f32)
        xin[0:F] = xc.transpose(2, 1, 0)
        xin[F:2 * F] = xc[:, ::-1, :].transpose(2, 1, 0)
        xin[2 * F] = 1.0
        xin = xin.reshape(2 * F + 1, T * BC)
        h0 = np.concatenate([np.asarray(kw["h0_fwd"], f32)[bs].T,
                             np.asarray(kw["h0_bwd"], f32)[bs].T], axis=0)
        m = dict(base)
        m["xin"] = np.ascontiguousarray(xin.astype(npdt))
        m["h0"] = np.ascontiguousarray(h0.astype(npdt))
        in_maps.append(m)
    return in_maps, split_l2


_CACHE = {}


def kernel(**inputs) -> np.ndarray:
    in_maps, split_l2 = prepare_maps(inputs)
    key = ("mod", split_l2)
    if key not in _CACHE:
        _CACHE[key] = build_module(split_l2)
    nc = _CACHE[key]
    res = run_bass_kernel_spmd(nc, in_maps, core_ids=list(range(NCORES)))
    outs = [r["out"] for r in res.results]  # each (P, BC)
    full = np.concatenate([o.T for o in outs], axis=0)  # (B, P)
    return np.ascontiguousarray(full.astype(np.float32))



# revision 9
# speedup vs baseline: 2.7489x; 2.7489x over previous
"""Trainium2 Bass kernel: 2-layer bidirectional GRU decoder + dense/softmax head.

Data-parallel over 8 NeuronCores (batch 4096 -> 512 per core). Inside each
core everything runs transposed: partition dim = hidden units with
[fwd(64); bwd(64)] packed to 128 partitions, batch on the free dim.

v3 design notes:
 - 4 chains of N=128 batch columns. The recurrence is latency-bound, so
   narrower per-step ops shorten the serial chain (sigmoid -> stt ->
   inject -> candidate -> update) that sets the per-step cadence.
 - One full 2KB PSUM bank per chain-step holds [zc | r | xh | rh]
   (4*128 fp32). A single start=True on the first matmul marks the
   bank's zero region; every later matmul into the bank uses
   start=False and relies on per-element pending-zero (first writer
   overwrites, repeat writers accumulate). One bank per chain-step
   means no cross-chain pending-zero hazards.
 - z-gate weights are negated host-side so sigmoid yields zc = 1-z.
   L1 (relu GRU): (1-z)*relu(w) == max((1-z)*w, 0) since 1-z > 0, so
   the relu disappears:   b = zc*xh  (DVE, PSUM read)
                          a = (zc-1)*h_prev (Pool, off critical path)
                          h' = max(b, 0) - a  (Pool stt)
 - L2 (tanh GRU): sigmoid and tanh are merged across chain pairs to
   halve activation-engine instruction overhead; per-pair PSUM banks
   [zc0|zc1|r0|r1] and [xh0|xh1|rh0|rh1].
 - xin is stored 3-row-group padded: step s lives at partitions
   32*(s%3) .. +19, column chunk s//3, so the input DMA spreads over
   83 partitions instead of 19 (per-partition DMA bandwidth limit).
"""

import os
import sys

sys.path.insert(0, "/opt/trn_rl_repo")

from contextlib import ExitStack

import numpy as np

import concourse.bass as bass
import concourse.bacc as bacc
import concourse.tile as tile
from concourse import mybir
from concourse.bass_utils import run_bass_kernel_spmd

AF = mybir.ActivationFunctionType
OP = mybir.AluOpType
DT = mybir.dt

B, T, F, H, DENSE, P = 4096, 72, 9, 64, 128, 24
NCORES = 8
BC = B // NCORES  # 512 batch per core
NF = 2 * F + 1  # 19: [x feats, reversed-x feats, ones]
TC = T // 3     # column chunks in the 3-row-group xin layout

N = 128          # batch columns per chain
NCH = BC // N    # 4 chains
NPAIR = NCH // 2

SDT = DT.bfloat16
NPDT = np.dtype("bfloat16")

# packed weight tensor column layout (all [128, x] bf16)
W_COLS = {
    "l1x": (0, 384),        # 3 gates x 128, replicated at 3 row groups
    "l1u": (384, 384),
    "l2a": (768, 384),
    "l2b": (1152, 384),
    "l2u": (1536, 384),
    "ident": (1920, 128),
    "dense_w": (2048, 128),
    "out_w": (2176, 32),    # [128, 24] padded to 32
}
WTOT = 2208

# packed fp32 vector tensor [128, NV]
V_COLS = {"br1h": 0, "br2h": 1, "bi2h": 2, "bz2n": 3, "br2": 4,
          "dense_b": 5, "out_b": 6}
NV = 7


def build_module(split_l2_sigmoid: bool, reps: int = 1):
    nc = bacc.Bacc("TRN2", target_bir_lowering=False, debug=False)

    # ---- DRAM I/O ----
    xin_cols = [BC, (TC // 2 - 1) * BC, (TC // 2) * BC]
    d_xin = [nc.dram_tensor(f"xin{i}", [128, xin_cols[i]], SDT,
                            kind="ExternalInput").ap() for i in range(3)]
    d_h0 = nc.dram_tensor("h0", [2 * H, BC], SDT, kind="ExternalInput").ap()
    d_w = nc.dram_tensor("wpack", [128, WTOT], SDT, kind="ExternalInput").ap()
    d_v = nc.dram_tensor("vpack", [128, NV], DT.float32,
                         kind="ExternalInput").ap()
    d_out = nc.dram_tensor("out", [P, BC], DT.float32, kind="ExternalOutput").ap()

    with tile.TileContext(nc) as tc, ExitStack() as ctx:
        wpool = ctx.enter_context(tc.tile_pool(name="weights", bufs=1))
        seq_pool = ctx.enter_context(tc.tile_pool(name="seq", bufs=1))
        spool = ctx.enter_context(tc.tile_pool(name="sig", bufs=3 * NCH))
        epool = ctx.enter_context(tc.tile_pool(name="ew", bufs=3 * NCH))
        hpool = ctx.enter_context(tc.tile_pool(name="h2", bufs=3 * NPAIR))
        fpool = ctx.enter_context(tc.tile_pool(name="feat", bufs=1))
        opool = ctx.enter_context(tc.tile_pool(name="outs", bufs=1))
        # every PSUM tile is one full 2KB bank; one shared tag so L1/L2/head
        # recycle the same 8 slots
        bank_ps = ctx.enter_context(tc.tile_pool(name="bank", bufs=8,
                                                 space="PSUM"))

        def psum_bank(name):
            return bank_ps.tile([128, 512], DT.float32, tag="bank", name=name)

        # ---- load weights (one packed DMA) + vectors + inputs ----
        w_t = wpool.tile([128, WTOT], SDT, tag="wpack")
        nc.sync.dma_start(w_t[:], d_w)

        def wsl(name):
            o, n = W_COLS[name]
            return w_t[:, o:o + n]

        v_t = wpool.tile([128, NV], DT.float32, tag="vpack")
        nc.sync.dma_start(v_t[:], d_v)

        def vsl(name, np_=128):
            return v_t[0:np_, V_COLS[name]:V_COLS[name] + 1]

        h0_t = wpool.tile([2 * H, BC], SDT, tag="h0t")
        nc.sync.dma_start(h0_t[:], d_h0)
        xin_t = [wpool.tile([128, xin_cols[i]], SDT, tag=f"xin{i}",
                            name=f"xin{i}")
                 for i in range(3)]
        nc.sync.dma_start(xin_t[0][:], d_xin[0])
        nc.sync.dma_start(xin_t[1][:], d_xin[1])
        nc.sync.dma_start(xin_t[2][:], d_xin[2])
        zeros_t = wpool.tile([2 * H, 2 * N], SDT, tag="zeros")
        nc.vector.memset(zeros_t[:], 0.0)

        def xin_sl(s, c):
            """[19, N] slice of xin for step s, chain c (row-group layout)."""
            g, tch = s % 3, s // 3
            if tch == 0:
                t_, col = xin_t[0], c * N
            elif tch < TC // 2:
                t_, col = xin_t[1], (tch - 1) * BC + c * N
            else:
                t_, col = xin_t[2], (tch - TC // 2) * BC + c * N
            return t_[32 * g: 32 * g + NF, col: col + N]

        def l1x_w(s, gate):
            g = s % 3
            o, _ = W_COLS["l1x"]
            gi = "zrh".index(gate)
            return w_t[32 * g: 32 * g + NF, o + gi * 128: o + (gi + 1) * 128]

        def uw(layer, gate):
            o, _ = W_COLS["l1u" if layer == 1 else "l2u"]
            gi = "zrh".index(gate)
            return w_t[:, o + gi * 128: o + (gi + 1) * 128]

        def l2w(which, gate):
            o, _ = W_COLS[which]  # l2a / l2b
            gi = "zrh".index(gate)
            return w_t[:, o + gi * 128: o + (gi + 1) * 128]

        # layer-1 output sequence, one tile per chain. Column block s holds
        # [h_fwd(time s); h_bwd(time T-1-s)] for that chain's batch columns.
        seq_t = [seq_pool.tile([2 * H, T * N], SDT, tag=f"seq{c}",
                               name=f"seq{c}")
                 for c in range(NCH)]

        def seq_sl(s, c):
            return seq_t[c][:, s * N: (s + 1) * N]

        feat = fpool.tile([2 * H, BC], SDT, tag="feat")

        rep_ctx = tc.For_i(0, reps, 1) if reps > 1 else None
        if rep_ctx is not None:
            rep_ctx.__enter__()

        def bank_mms(specs):
            """Emit matmuls into one PSUM bank: single start on the first,
            stop on the last; intermediate writers rely on per-element
            pending-zero (first write of a byte replaces, later adds)."""
            last = len(specs) - 1
            for i, (out_ap, lhsT, rhs) in enumerate(specs):
                nc.tensor.matmul(out_ap, lhsT, rhs, start=(i == 0),
                                 stop=(i == last), skip_group_check=True)

        # ---------------- layer 1 (relu GRU), per-chain ----------------
        def l1_step(s, c, h_prev, h_out):
            ps = psum_bank(f"l1ps_{s}_{c}")
            xw = xin_sl(s, c)
            bank_mms([
                (ps[:, 0:N], l1x_w(s, "z"), xw),
                (ps[:, N:2 * N], l1x_w(s, "r"), xw),
                (ps[:, 0:N], uw(1, "z"), h_prev),
                (ps[:, N:2 * N], uw(1, "r"), h_prev),
                (ps[:, 2 * N:3 * N], l1x_w(s, "h"), xw),
                (ps[:, 3 * N:4 * N], uw(1, "h"), h_prev),
            ])
            sg = spool.tile([2 * H, 2 * N], SDT, tag="sg")
            nc.scalar.activation(sg[:], ps[:, 0:2 * N], AF.Sigmoid)
            t = epool.tile([2 * H, N], SDT, tag="t")
            nc.vector.scalar_tensor_tensor(t[:], ps[:, 3 * N:4 * N],
                                           vsl("br1h"), sg[:, N:2 * N],
                                           OP.add, OP.mult)
            nc.tensor.matmul(ps[:, 2 * N:3 * N], wsl("ident"), t[:],
                             start=False, stop=True, skip_group_check=True)
            # b = zc*relu(w) == max(zc*w, 0) since zc = 1-z > 0; the relu
            # folds into the PSUM-reading stt. Tail is plain tensor_tensor
            # (Pool supports tt but not stt on real hw):
            #   h' = b - (zc*h - h) = (1-z)*relu(w) + z*h
            b = epool.tile([2 * H, N], SDT, tag="b")
            nc.vector.scalar_tensor_tensor(b[:], ps[:, 2 * N:3 * N], 0.0,
                                           sg[:, 0:N], OP.max, OP.mult)
            p1 = epool.tile([2 * H, N], SDT, tag="p1")
            nc.gpsimd.tensor_mul(p1[:], sg[:, 0:N], h_prev)
            an = epool.tile([2 * H, N], SDT, tag="an")
            nc.gpsimd.tensor_sub(an[:], p1[:], h_prev)
            nc.gpsimd.tensor_sub(h_out, b[:], an[:])

        for s in range(T):
            for c in range(NCH):
                h_prev = (h0_t[:, c * N: c * N + N] if s == 0
                          else seq_sl(s - 1, c))
                l1_step(s, c, h_prev, seq_sl(s, c))

        # ---------------- layer 2 (tanh GRU), chain-pair merged ----------
        h2_prev = {p: None for p in range(NPAIR)}  # merged [128, 2N] tiles

        def l2_step(s, p, h_prev, h_out):
            """One step for chain pair p (chains 2p, 2p+1). h_prev/h_out are
            merged [128, 2N]; None h_prev means zero initial state."""
            c0, c1 = 2 * p, 2 * p + 1
            x1 = [seq_sl(s, c) for c in (c0, c1)]
            x2 = [seq_sl(T - 1 - s, c) for c in (c0, c1)]
            zr = psum_bank(f"l2zr_{s}_{p}")
            hx = psum_bank(f"l2hx_{s}_{p}")
            specs = []
            for gi, gate in enumerate(("z", "r")):
                for i in (0, 1):
                    specs.append((zr[:, (2 * gi + i) * N:(2 * gi + i + 1) * N],
                                  l2w("l2a", gate), x1[i]))
                    specs.append((zr[:, (2 * gi + i) * N:(2 * gi + i + 1) * N],
                                  l2w("l2b", gate), x2[i]))
                if h_prev is not None:
                    for i in (0, 1):
                        specs.append((zr[:, (2 * gi + i) * N:(2 * gi + i + 1) * N],
                                      uw(2, gate), h_prev[:, i * N:(i + 1) * N]))
            bank_mms(specs)
            hx_specs = []
            for i in (0, 1):
                hx_specs.append((hx[:, i * N:(i + 1) * N], l2w("l2a", "h"), x1[i]))
                hx_specs.append((hx[:, i * N:(i + 1) * N], l2w("l2b", "h"), x2[i]))
            if h_prev is not None:
                for i in (0, 1):
                    hx_specs.append((hx[:, (2 + i) * N:(3 + i) * N], uw(2, "h"),
                                     h_prev[:, i * N:(i + 1) * N]))
            last = len(hx_specs) - 1
            for i, (out_ap, lhsT, rhs) in enumerate(hx_specs):
                nc.tensor.matmul(out_ap, lhsT, rhs, start=(i == 0), stop=False,
                                 skip_group_check=True)
            # merged sigmoid over [zc0|zc1|r0|r1]; r first (it gates the
            # stt on the serial chain), zc after (only needed by the tail)
            sg = spool.tile([2 * H, 4 * N], SDT, tag="sg2", name=f"sg2_{s}_{p}")
            if split_l2_sigmoid:
                nc.scalar.activation(sg[:, 2 * N:4 * N], zr[:, 2 * N:4 * N],
                                     AF.Sigmoid, bias=vsl("br2"))
                nc.scalar.activation(sg[:, 0:2 * N], zr[:, 0:2 * N], AF.Sigmoid,
                                     bias=vsl("bz2n"))
            else:
                nc.scalar.activation(sg[:, 2 * N:4 * N], zr[:, 2 * N:4 * N],
                                     AF.Sigmoid)
                nc.scalar.activation(sg[:, 0:2 * N], zr[:, 0:2 * N],
                                     AF.Sigmoid)
            # per-chain: t_i = (rh_i + br2h) * r_i ; inject into xh_i
            for i in (0, 1):
                t = epool.tile([2 * H, N], SDT, tag="t")
                if h_prev is not None:
                    nc.vector.scalar_tensor_tensor(
                        t[:], hx[:, (2 + i) * N:(3 + i) * N], vsl("br2h"),
                        sg[:, (2 + i) * N:(3 + i) * N], OP.add, OP.mult)
                else:
                    # rh == 0: t = br2h * r
                    nc.vector.tensor_scalar(t[:],
                                            sg[:, (2 + i) * N:(3 + i) * N],
                                            vsl("br2h"), None, OP.mult)
                nc.tensor.matmul(hx[:, i * N:(i + 1) * N], wsl("ident"), t[:],
                                 start=False, stop=(i == 1),
                                 skip_group_check=True)
            # merged tanh candidate over [xh0|xh1]
            hh = epool.tile([2 * H, 2 * N], SDT, tag="hh2", name=f"hh2_{s}_{p}")
            nc.scalar.activation(hh[:], hx[:, 0:2 * N], AF.Tanh,
                                 bias=vsl("bi2h"))
            # h' = zc*hh + z*h = q - a;  q = zc*hh (DVE), a = (zc-1)*h (Pool)
            q = epool.tile([2 * H, 2 * N], SDT, tag="q2", name=f"q2_{s}_{p}")
            nc.vector.tensor_mul(q[:], sg[:, 0:2 * N], hh[:])
            if h_prev is None:
                nc.gpsimd.tensor_copy(h_out, q[:])
            else:
                p1 = epool.tile([2 * H, 2 * N], SDT, tag="p12",
                                name=f"p12_{s}_{p}")
                nc.gpsimd.tensor_mul(p1[:], sg[:, 0:2 * N], h_prev)
                an = epool.tile([2 * H, 2 * N], SDT, tag="an2",
                                name=f"an2_{s}_{p}")
                nc.gpsimd.tensor_sub(an[:], p1[:], h_prev)
                nc.gpsimd.tensor_sub(h_out, q[:], an[:])

        for s in range(T):
            for p in range(NPAIR):
                if s == T - 1:
                    h_out = feat[:, p * 2 * N:(p + 1) * 2 * N]
                else:
                    h2t = hpool.tile([2 * H, 2 * N], SDT, tag="h2t")
                    h_out = h2t[:]
                l2_step(s, p, h2_prev[p], h_out)
                h2_prev[p] = h_out

        # ---- head: relu(feat @ dense_W + b) -> softmax(out_W + b) ----
        ps_d = psum_bank("ps_d")
        nc.tensor.matmul(ps_d[:, 0:BC], wsl("dense_w"), feat[:],
                         start=True, stop=True)
        h3 = opool.tile([DENSE, BC], SDT, tag="h3")
        nc.scalar.activation(h3[:], ps_d[:, 0:BC], AF.Relu, bias=vsl("dense_b"))

        ps_l = psum_bank("ps_l")
        ow_o = W_COLS["out_w"][0]
        nc.tensor.matmul(ps_l[0:P, 0:BC], w_t[:, ow_o:ow_o + P], h3[:],
                         start=True, stop=True)
        ex = opool.tile([P, BC], DT.float32, tag="ex")
        nc.scalar.activation(ex[:], ps_l[0:P, 0:BC], AF.Exp,
                             bias=vsl("out_b", np_=P))

        ones_a = opool.tile([P, 1], DT.float32, tag="ones_a2")
        nc.vector.memset(ones_a[:], 1.0)
        ps_s = psum_bank("ps_s")
        nc.tensor.matmul(ps_s[0:1, 0:BC], ones_a[:], ex[:],
                         start=True, stop=True)
        rin = opool.tile([1, BC], DT.float32, tag="rin")
        nc.vector.reciprocal(rin[:], ps_s[0:1, 0:BC])
        ones_b = opool.tile([1, P], DT.float32, tag="ones_b2")
        nc.vector.memset(ones_b[:], 1.0)
        ps_b = psum_bank("ps_b")
        nc.tensor.matmul(ps_b[0:P, 0:BC], ones_b[:], rin[:],
                         start=True, stop=True)
        res = opool.tile([P, BC], DT.float32, tag="res")
        nc.vector.tensor_mul(res[:], ex[:], ps_b[0:P, 0:BC])
        nc.sync.dma_start(d_out, res[:])

        if rep_ctx is not None:
            rep_ctx.__exit__(None, None, None)

    nc.finalize()
    return nc


def _gs(g):
    i = "zrh".index(g)
    return slice(i * H, (i + 1) * H)


def prepare_maps(kw):
    """Host-side prep: build per-core input maps (numpy only).

    z-gate weights/biases are negated so sigmoid yields zc = 1-z.
    """
    f32 = np.float32
    x = np.asarray(kw["x"], f32)

    def gsign(g):
        return -1.0 if g == "z" else 1.0

    # ---- packed weights [128, WTOT] ----
    wpack = np.zeros((128, WTOT), f32)

    # l1x: [19, 384] replicated at 3 row groups
    l1x19 = np.zeros((NF, 384), f32)
    for gi, g in enumerate("zrh"):
        gs = _gs(g)
        blk = np.zeros((NF, 2 * H), f32)
        blk[0:F, 0:H] = kw["d1f_W"][:, gs]
        blk[F:2 * F, H:2 * H] = kw["d1b_W"][:, gs]
        bias_f = kw["d1f_bi"][gs] + (kw["d1f_br"][gs] if g != "h" else 0.0)
        bias_b = kw["d1b_bi"][gs] + (kw["d1b_br"][gs] if g != "h" else 0.0)
        blk[2 * F, 0:H] = bias_f
        blk[2 * F, H:2 * H] = bias_b
        l1x19[:, gi * 128:(gi + 1) * 128] = gsign(g) * blk
    o, _ = W_COLS["l1x"]
    for grp in range(3):
        wpack[32 * grp: 32 * grp + NF, o:o + 384] = l1x19

    for gi, g in enumerate("zrh"):
        gs = _gs(g)
        sgn = gsign(g)
        for name, uf, ub in (("l1u", kw["d1f_U"], kw["d1b_U"]),
                             ("l2u", kw["d2f_U"], kw["d2b_U"])):
            o, _ = W_COLS[name]
            wpack[0:H, o + gi * 128: o + gi * 128 + H] = sgn * uf[:, gs]
            wpack[H:2 * H, o + gi * 128 + H: o + (gi + 1) * 128] = sgn * ub[:, gs]
        o, _ = W_COLS["l2a"]
        wpack[0:H, o + gi * 128: o + gi * 128 + H] = sgn * kw["d2f_W"][0:H, gs]
        wpack[H:2 * H, o + gi * 128 + H: o + (gi + 1) * 128] = \
            sgn * kw["d2b_W"][H:2 * H, gs]
        o, _ = W_COLS["l2b"]
        wpack[0:H, o + gi * 128 + H: o + (gi + 1) * 128] = \
            sgn * kw["d2b_W"][0:H, gs]
        wpack[H:2 * H, o + gi * 128: o + gi * 128 + H] = \
            sgn * kw["d2f_W"][H:2 * H, gs]

    o, _ = W_COLS["ident"]
    wpack[:, o:o + 128] = np.eye(2 * H, dtype=f32)
    o, _ = W_COLS["dense_w"]
    wpack[:, o:o + 128] = np.asarray(kw["dense_W"], f32)
    o, _ = W_COLS["out_w"]
    wpack[:, o:o + P] = np.asarray(kw["out_W"], f32)

    # ---- packed fp32 vectors [128, NV] ----
    vpack = np.zeros((128, NV), f32)
    gh, gz, gr = _gs("h"), _gs("z"), _gs("r")
    vpack[:, V_COLS["br1h"]] = np.concatenate([kw["d1f_br"][gh], kw["d1b_br"][gh]])
    vpack[:, V_COLS["br2h"]] = np.concatenate([kw["d2f_br"][gh], kw["d2b_br"][gh]])
    vpack[:, V_COLS["bi2h"]] = np.concatenate([kw["d2f_bi"][gh], kw["d2b_bi"][gh]])
    vpack[:, V_COLS["bz2n"]] = -np.concatenate(
        [kw["d2f_bi"][gz] + kw["d2f_br"][gz], kw["d2b_bi"][gz] + kw["d2b_br"][gz]])
    vpack[:, V_COLS["br2"]] = np.concatenate(
        [kw["d2f_bi"][gr] + kw["d2f_br"][gr], kw["d2b_bi"][gr] + kw["d2b_br"][gr]])
    vpack[:, V_COLS["dense_b"]] = np.asarray(kw["dense_b"], f32)
    vpack[0:P, V_COLS["out_b"]] = np.asarray(kw["out_b"], f32)
    split_l2 = bool(np.any(vpack[:, V_COLS["bz2n"]]) or
                    np.any(vpack[:, V_COLS["br2"]]))

    base = {
        "wpack": np.ascontiguousarray(wpack.astype(NPDT)),
        "vpack": np.ascontiguousarray(vpack),
    }

    in_maps = []
    for cid in range(NCORES):
        bs = slice(cid * BC, (cid + 1) * BC)
        xc = x[bs]  # (BC, T, F)
        xin19 = np.empty((NF, T, BC), f32)
        xin19[0:F] = xc.transpose(2, 1, 0)
        xin19[F:2 * F] = xc[:, ::-1, :].transpose(2, 1, 0)
        xin19[2 * F] = 1.0
        # 3-row-group layout: step s -> rows 32*(s%3), col chunk s//3
        xin32 = np.zeros((128, TC * BC), f32)
        for grp in range(3):
            xin32[32 * grp: 32 * grp + NF] = \
                xin19[:, grp::3, :].reshape(NF, TC * BC)
        xin32 = xin32.astype(NPDT)
        h0 = np.concatenate([np.asarray(kw["h0_fwd"], f32)[bs].T,
                             np.asarray(kw["h0_bwd"], f32)[bs].T], axis=0)
        m = dict(base)
        half = (TC // 2) * BC
        m["xin0"] = np.ascontiguousarray(xin32[:, :BC])
        m["xin1"] = np.ascontiguousarray(xin32[:, BC:half])
        m["xin2"] = np.ascontiguousarray(xin32[:, half:])
        m["h0"] = np.ascontiguousarray(h0.astype(NPDT))
        in_maps.append(m)
    return in_maps, split_l2


_CACHE = {}


def kernel(**inputs) -> np.ndarray:
    in_maps, split_l2 = prepare_maps(inputs)
    key = ("mod", split_l2)
    if key not in _CACHE:
        _CACHE[key] = build_module(split_l2)
    nc = _CACHE[key]
    res = run_bass_kernel_spmd(nc, in_maps, core_ids=list(range(NCORES)))
    outs = [r["out"] for r in res.results]  # each (P, BC)
    full = np.concatenate([o.T for o in outs], axis=0)  # (B, P)
    return np.ascontiguousarray(full.astype(np.float32))


# revision 11
# speedup vs baseline: 3.3285x; 1.2109x over previous
"""Trainium2 Bass kernel: 2-layer bidirectional GRU decoder + dense/softmax head.

Data-parallel over 8 NeuronCores (batch 4096 -> 512 per core). Inside each
core everything runs transposed: partition dim = hidden units with
[fwd(64); bwd(64)] packed to 128 partitions, batch on the free dim.

v3 design notes:
 - 4 chains of N=128 batch columns. The recurrence is latency-bound, so
   narrower per-step ops shorten the serial chain (sigmoid -> stt ->
   inject -> candidate -> update) that sets the per-step cadence.
 - One full 2KB PSUM bank per chain-step holds [zc | r | xh | rh]
   (4*128 fp32). A single start=True on the first matmul marks the
   bank's zero region; every later matmul into the bank uses
   start=False and relies on per-element pending-zero (first writer
   overwrites, repeat writers accumulate). One bank per chain-step
   means no cross-chain pending-zero hazards.
 - z-gate weights are negated host-side so sigmoid yields zc = 1-z.
   L1 (relu GRU): (1-z)*relu(w) == max((1-z)*w, 0) since 1-z > 0, so
   the relu disappears:   b = zc*xh  (DVE, PSUM read)
                          a = (zc-1)*h_prev (Pool, off critical path)
                          h' = max(b, 0) - a  (Pool stt)
 - L2 (tanh GRU): sigmoid and tanh are merged across chain pairs to
   halve activation-engine instruction overhead; per-pair PSUM banks
   [zc0|zc1|r0|r1] and [xh0|xh1|rh0|rh1].
 - xin is stored 3-row-group padded: step s lives at partitions
   32*(s%3) .. +19, column chunk s//3, so the input DMA spreads over
   83 partitions instead of 19 (per-partition DMA bandwidth limit).
"""

import os
import sys

sys.path.insert(0, "/opt/trn_rl_repo")

from contextlib import ExitStack

import numpy as np

import concourse.bass as bass
import concourse.bacc as bacc
import concourse.tile as tile
from concourse import mybir
from concourse.bass_utils import run_bass_kernel_spmd

AF = mybir.ActivationFunctionType
OP = mybir.AluOpType
DT = mybir.dt

B, T, F, H, DENSE, P = 4096, 72, 9, 64, 128, 24
NCORES = 8
BC = B // NCORES  # 512 batch per core
NF = 2 * F + 1  # 19: [x feats, reversed-x feats, ones]
TC = T // 3     # column chunks in the 3-row-group xin layout

N = 128          # batch columns per chain
NCH = BC // N    # 4 chains
NPAIR = NCH // 2

SDT = DT.bfloat16
NPDT = np.dtype("bfloat16")

# packed weight tensor column layout (all [128, x] bf16)
W_COLS = {
    "l1x": (0, 384),        # 3 gates x 128, replicated at 3 row groups
    "l1u": (384, 384),
    "l2a": (768, 384),
    "l2b": (1152, 384),
    "l2u": (1536, 384),
    "ident": (1920, 128),
    "dense_w": (2048, 128),
    "out_w": (2176, 32),    # [128, 24] padded to 32
}
WTOT = 2208

# packed fp32 vector tensor [128, NV]
V_COLS = {"br1h": 0, "br2h": 1, "bi2h": 2, "bz2n": 3, "br2": 4,
          "dense_b": 5, "out_b": 6}
NV = 7


def build_module(split_l2_sigmoid: bool, reps: int = 1):
    nc = bacc.Bacc("TRN2", target_bir_lowering=False, debug=False)

    # ---- DRAM I/O ----
    xin_cols = [BC, (TC // 2 - 1) * BC, (TC // 2) * BC]
    d_xin = [nc.dram_tensor(f"xin{i}", [128, xin_cols[i]], SDT,
                            kind="ExternalInput").ap() for i in range(3)]
    d_h0 = nc.dram_tensor("h0", [2 * H, BC], SDT, kind="ExternalInput").ap()
    d_w = nc.dram_tensor("wpack", [128, WTOT], SDT, kind="ExternalInput").ap()
    d_v = nc.dram_tensor("vpack", [128, NV], DT.float32,
                         kind="ExternalInput").ap()
    d_out = nc.dram_tensor("out", [P, BC], DT.float32, kind="ExternalOutput").ap()

    with tile.TileContext(nc) as tc, ExitStack() as ctx:
        wpool = ctx.enter_context(tc.tile_pool(name="weights", bufs=1))
        seq_pool = ctx.enter_context(tc.tile_pool(name="seq", bufs=1))
        spool = ctx.enter_context(tc.tile_pool(name="sig", bufs=3 * NCH))
        epool = ctx.enter_context(tc.tile_pool(name="ew", bufs=3 * NCH))
        hpool = ctx.enter_context(tc.tile_pool(name="h2", bufs=3 * NPAIR))
        fpool = ctx.enter_context(tc.tile_pool(name="feat", bufs=1))
        opool = ctx.enter_context(tc.tile_pool(name="outs", bufs=1))
        # every PSUM tile is one full 2KB bank; one shared tag so L1/L2/head
        # recycle the same 8 slots
        bank_ps = ctx.enter_context(tc.tile_pool(name="bank", bufs=8,
                                                 space="PSUM"))

        def psum_bank(name):
            return bank_ps.tile([128, 512], DT.float32, tag="bank", name=name)

        # ---- load weights (one packed DMA) + vectors + inputs ----
        w_t = wpool.tile([128, WTOT], SDT, tag="wpack")
        nc.sync.dma_start(w_t[:], d_w)

        def wsl(name):
            o, n = W_COLS[name]
            return w_t[:, o:o + n]

        v_t = wpool.tile([128, NV], DT.float32, tag="vpack")
        nc.sync.dma_start(v_t[:], d_v)

        def vsl(name, np_=128):
            return v_t[0:np_, V_COLS[name]:V_COLS[name] + 1]

        h0_t = wpool.tile([2 * H, BC], SDT, tag="h0t")
        nc.sync.dma_start(h0_t[:], d_h0)
        xin_t = [wpool.tile([128, xin_cols[i]], SDT, tag=f"xin{i}",
                            name=f"xin{i}")
                 for i in range(3)]
        nc.sync.dma_start(xin_t[0][:], d_xin[0])
        nc.sync.dma_start(xin_t[1][:], d_xin[1])
        nc.sync.dma_start(xin_t[2][:], d_xin[2])
        zeros_t = wpool.tile([2 * H, 2 * N], SDT, tag="zeros")
        nc.vector.memset(zeros_t[:], 0.0)

        def xin_sl(s, c):
            """[19, N] slice of xin for step s, chain c (row-group layout)."""
            g, tch = s % 3, s // 3
            if tch == 0:
                t_, col = xin_t[0], c * N
            elif tch < TC // 2:
                t_, col = xin_t[1], (tch - 1) * BC + c * N
            else:
                t_, col = xin_t[2], (tch - TC // 2) * BC + c * N
            return t_[32 * g: 32 * g + NF, col: col + N]

        def l1x_w(s, gate):
            g = s % 3
            o, _ = W_COLS["l1x"]
            gi = "zrh".index(gate)
            return w_t[32 * g: 32 * g + NF, o + gi * 128: o + (gi + 1) * 128]

        def uw(layer, gate):
            o, _ = W_COLS["l1u" if layer == 1 else "l2u"]
            gi = "zrh".index(gate)
            return w_t[:, o + gi * 128: o + (gi + 1) * 128]

        def l2w(which, gate):
            o, _ = W_COLS[which]  # l2a / l2b
            gi = "zrh".index(gate)
            return w_t[:, o + gi * 128: o + (gi + 1) * 128]

        QUAD = False  # K=64 quadrant pairs compile but fault at runtime
        # (PE col-group tiling hits the quadrant-3 XBUS hw bug); keep off

        def l2x_specs(gate, x1, x2, out_ap):
            """x-projection matmuls for one gate/chain: l2a@x1 + l2b@x2.
            With QUAD, each half-zero 128x128 block becomes two 64x64
            tile_position quadrants; the four quadrants tile the whole PE
            array and run concurrently on hardware."""
            wa, wb = l2w("l2a", gate), l2w("l2b", gate)
            if not QUAD:
                return [(out_ap, wa, x1), (out_ap, wb, x2)]
            return [
                (out_ap[0:H, :], wa[0:H, 0:H], x1[0:H, :]),
                (out_ap[H:2 * H, :], wa[H:2 * H, H:2 * H], x1[H:2 * H, :]),
                (out_ap[H:2 * H, :], wb[0:H, H:2 * H], x2[0:H, :]),
                (out_ap[0:H, :], wb[H:2 * H, 0:H], x2[H:2 * H, :]),
            ]

        # layer-1 output sequence, one tile per chain. Column block s holds
        # [h_fwd(time s); h_bwd(time T-1-s)] for that chain's batch columns.
        seq_t = [seq_pool.tile([2 * H, T * N], SDT, tag=f"seq{c}",
                               name=f"seq{c}")
                 for c in range(NCH)]

        def seq_sl(s, c):
            return seq_t[c][:, s * N: (s + 1) * N]

        feat = fpool.tile([2 * H, BC], SDT, tag="feat")

        rep_ctx = tc.For_i(0, reps, 1) if reps > 1 else None
        if rep_ctx is not None:
            rep_ctx.__enter__()

        def bank_mms(specs, stop_last=True):
            """Emit matmuls into one PSUM bank: start=True on the first
            writer of each (bank, partition-range) so its zero region gets
            marked; later writers rely on per-element pending-zero (first
            write of a byte replaces, later adds)."""
            seen = set()
            last = len(specs) - 1
            for i, (out_ap, lhsT, rhs) in enumerate(specs):
                key = (id(out_ap.tensor), out_ap.base_partition())
                nc.tensor.matmul(out_ap, lhsT, rhs,
                                 start=key not in seen,
                                 stop=(i == last and stop_last),
                                 skip_group_check=True)
                seen.add(key)

        # ---------------- layer 1 (relu GRU), per-chain ----------------
        def l1_step(s, c, h_prev, h_out):
            ps = psum_bank(f"l1ps_{s}_{c}")
            xw = xin_sl(s, c)
            bank_mms([
                (ps[:, 0:N], l1x_w(s, "z"), xw),
                (ps[:, N:2 * N], l1x_w(s, "r"), xw),
                (ps[:, 0:N], uw(1, "z"), h_prev),
                (ps[:, N:2 * N], uw(1, "r"), h_prev),
                (ps[:, 2 * N:3 * N], l1x_w(s, "h"), xw),
                (ps[:, 3 * N:4 * N], uw(1, "h"), h_prev),
            ])
            sg = spool.tile([2 * H, 2 * N], SDT, tag="sg")
            nc.scalar.activation(sg[:], ps[:, 0:2 * N], AF.Sigmoid)
            t = epool.tile([2 * H, N], SDT, tag="t")
            nc.vector.scalar_tensor_tensor(t[:], ps[:, 3 * N:4 * N],
                                           vsl("br1h"), sg[:, N:2 * N],
                                           OP.add, OP.mult)
            nc.tensor.matmul(ps[:, 2 * N:3 * N], wsl("ident"), t[:],
                             start=False, stop=True, skip_group_check=True)
            # b = zc*relu(w) == max(zc*w, 0) since zc = 1-z > 0; the relu
            # folds into the PSUM-reading stt. Tail is plain tensor_tensor
            # (Pool supports tt but not stt on real hw):
            #   h' = b - (zc*h - h) = (1-z)*relu(w) + z*h
            b = epool.tile([2 * H, N], SDT, tag="b")
            nc.vector.scalar_tensor_tensor(b[:], ps[:, 2 * N:3 * N], 0.0,
                                           sg[:, 0:N], OP.max, OP.mult)
            p1 = epool.tile([2 * H, N], SDT, tag="p1")
            nc.gpsimd.tensor_mul(p1[:], sg[:, 0:N], h_prev)
            an = epool.tile([2 * H, N], SDT, tag="an")
            nc.gpsimd.tensor_sub(an[:], p1[:], h_prev)
            nc.gpsimd.tensor_sub(h_out, b[:], an[:])

        for s in range(T):
            for c in range(NCH):
                h_prev = (h0_t[:, c * N: c * N + N] if s == 0
                          else seq_sl(s - 1, c))
                l1_step(s, c, h_prev, seq_sl(s, c))

        # ---------------- layer 2 (tanh GRU), chain-pair merged ----------
        h2_prev = {p: None for p in range(NPAIR)}  # merged [128, 2N] tiles

        def l2_step(s, p, h_prev, h_out):
            """One step for chain pair p (chains 2p, 2p+1). h_prev/h_out are
            merged [128, 2N]; None h_prev means zero initial state."""
            c0, c1 = 2 * p, 2 * p + 1
            x1 = [seq_sl(s, c) for c in (c0, c1)]
            x2 = [seq_sl(T - 1 - s, c) for c in (c0, c1)]
            zr = psum_bank(f"l2zr_{s}_{p}")
            hx = psum_bank(f"l2hx_{s}_{p}")
            specs = []
            for gi, gate in enumerate(("z", "r")):
                for i in (0, 1):
                    out_ap = zr[:, (2 * gi + i) * N:(2 * gi + i + 1) * N]
                    specs.extend(l2x_specs(gate, x1[i], x2[i], out_ap))
                if h_prev is not None:
                    for i in (0, 1):
                        specs.append((zr[:, (2 * gi + i) * N:(2 * gi + i + 1) * N],
                                      uw(2, gate), h_prev[:, i * N:(i + 1) * N]))
            bank_mms(specs)
            hx_specs = []
            for i in (0, 1):
                hx_specs.extend(l2x_specs("h", x1[i], x2[i],
                                          hx[:, i * N:(i + 1) * N]))
            if h_prev is not None:
                for i in (0, 1):
                    hx_specs.append((hx[:, (2 + i) * N:(3 + i) * N], uw(2, "h"),
                                     h_prev[:, i * N:(i + 1) * N]))
            bank_mms(hx_specs, stop_last=False)
            # merged sigmoid over [zc0|zc1|r0|r1]; r first (it gates the
            # stt on the serial chain), zc after (only needed by the tail)
            sg = spool.tile([2 * H, 4 * N], SDT, tag="sg2", name=f"sg2_{s}_{p}")
            if split_l2_sigmoid:
                nc.scalar.activation(sg[:, 2 * N:4 * N], zr[:, 2 * N:4 * N],
                                     AF.Sigmoid, bias=vsl("br2"))
                nc.scalar.activation(sg[:, 0:2 * N], zr[:, 0:2 * N], AF.Sigmoid,
                                     bias=vsl("bz2n"))
            else:
                nc.scalar.activation(sg[:, 2 * N:4 * N], zr[:, 2 * N:4 * N],
                                     AF.Sigmoid)
                nc.scalar.activation(sg[:, 0:2 * N], zr[:, 0:2 * N],
                                     AF.Sigmoid)
            # per-chain: t_i = (rh_i + br2h) * r_i ; inject into xh_i
            for i in (0, 1):
                t = epool.tile([2 * H, N], SDT, tag="t")
                if h_prev is not None:
                    nc.vector.scalar_tensor_tensor(
                        t[:], hx[:, (2 + i) * N:(3 + i) * N], vsl("br2h"),
                        sg[:, (2 + i) * N:(3 + i) * N], OP.add, OP.mult)
                else:
                    # rh == 0: t = br2h * r
                    nc.vector.tensor_scalar(t[:],
                                            sg[:, (2 + i) * N:(3 + i) * N],
                                            vsl("br2h"), None, OP.mult)
                nc.tensor.matmul(hx[:, i * N:(i + 1) * N], wsl("ident"), t[:],
                                 start=False, stop=(i == 1),
                                 skip_group_check=True)
            # merged tanh candidate over [xh0|xh1]
            hh = epool.tile([2 * H, 2 * N], SDT, tag="hh2", name=f"hh2_{s}_{p}")
            nc.scalar.activation(hh[:], hx[:, 0:2 * N], AF.Tanh,
                                 bias=vsl("bi2h"))
            # h' = zc*hh + z*h = q - a;  q = zc*hh (DVE), a = (zc-1)*h (Pool)
            q = epool.tile([2 * H, 2 * N], SDT, tag="q2", name=f"q2_{s}_{p}")
            nc.vector.tensor_mul(q[:], sg[:, 0:2 * N], hh[:])
            if h_prev is None:
                nc.gpsimd.tensor_copy(h_out, q[:])
            else:
                p1 = epool.tile([2 * H, 2 * N], SDT, tag="p12",
                                name=f"p12_{s}_{p}")
                nc.gpsimd.tensor_mul(p1[:], sg[:, 0:2 * N], h_prev)
                an = epool.tile([2 * H, 2 * N], SDT, tag="an2",
                                name=f"an2_{s}_{p}")
                nc.gpsimd.tensor_sub(an[:], p1[:], h_prev)
                nc.gpsimd.tensor_sub(h_out, q[:], an[:])

        for s in range(T):
            for p in range(NPAIR):
                if s == T - 1:
                    h_out = feat[:, p * 2 * N:(p + 1) * 2 * N]
                else:
                    h2t = hpool.tile([2 * H, 2 * N], SDT, tag="h2t")
                    h_out = h2t[:]
                l2_step(s, p, h2_prev[p], h_out)
                h2_prev[p] = h_out

        # ---- head: relu(feat @ dense_W + b) -> softmax(out_W + b) ----
        ps_d = psum_bank("ps_d")
        nc.tensor.matmul(ps_d[:, 0:BC], wsl("dense_w"), feat[:],
                         start=True, stop=True)
        h3 = opool.tile([DENSE, BC], SDT, tag="h3")
        nc.scalar.activation(h3[:], ps_d[:, 0:BC], AF.Relu, bias=vsl("dense_b"))

        ps_l = psum_bank("ps_l")
        ow_o = W_COLS["out_w"][0]
        nc.tensor.matmul(ps_l[0:P, 0:BC], w_t[:, ow_o:ow_o + P], h3[:],
                         start=True, stop=True)
        ex = opool.tile([P, BC], DT.float32, tag="ex")
        nc.scalar.activation(ex[:], ps_l[0:P, 0:BC], AF.Exp,
                             bias=vsl("out_b", np_=P))

        ones_a = opool.tile([P, 1], DT.float32, tag="ones_a2")
        nc.vector.memset(ones_a[:], 1.0)
        ps_s = psum_bank("ps_s")
        nc.tensor.matmul(ps_s[0:1, 0:BC], ones_a[:], ex[:],
                         start=True, stop=True)
        rin = opool.tile([1, BC], DT.float32, tag="rin")
        nc.vector.reciprocal(rin[:], ps_s[0:1, 0:BC])
        ones_b = opool.tile([1, P], DT.float32, tag="ones_b2")
        nc.vector.memset(ones_b[:], 1.0)
        ps_b = psum_bank("ps_b")
        nc.tensor.matmul(ps_b[0:P, 0:BC], ones_b[:], rin[:],
                         start=True, stop=True)
        res = opool.tile([P, BC], DT.float32, tag="res")
        nc.vector.tensor_mul(res[:], ex[:], ps_b[0:P, 0:BC])
        nc.sync.dma_start(d_out, res[:])

        if rep_ctx is not None:
            rep_ctx.__exit__(None, None, None)

    nc.finalize()
    return nc


def _gs(g):
    i = "zrh".index(g)
    return slice(i * H, (i + 1) * H)


def prepare_maps(kw):
    """Host-side prep: build per-core input maps (numpy only).

    z-gate weights/biases are negated so sigmoid yields zc = 1-z.
    """
    f32 = np.float32
    x = np.asarray(kw["x"], f32)

    def gsign(g):
        return -1.0 if g == "z" else 1.0

    # ---- packed weights [128, WTOT] ----
    wpack = np.zeros((128, WTOT), f32)

    # l1x: [19, 384] replicated at 3 row groups
    l1x19 = np.zeros((NF, 384), f32)
    for gi, g in enumerate("zrh"):
        gs = _gs(g)
        blk = np.zeros((NF, 2 * H), f32)
        blk[0:F, 0:H] = kw["d1f_W"][:, gs]
        blk[F:2 * F, H:2 * H] = kw["d1b_W"][:, gs]
        bias_f = kw["d1f_bi"][gs] + (kw["d1f_br"][gs] if g != "h" else 0.0)
        bias_b = kw["d1b_bi"][gs] + (kw["d1b_br"][gs] if g != "h" else 0.0)
        blk[2 * F, 0:H] = bias_f
        blk[2 * F, H:2 * H] = bias_b
        l1x19[:, gi * 128:(gi + 1) * 128] = gsign(g) * blk
    o, _ = W_COLS["l1x"]
    for grp in range(3):
        wpack[32 * grp: 32 * grp + NF, o:o + 384] = l1x19

    for gi, g in enumerate("zrh"):
        gs = _gs(g)
        sgn = gsign(g)
        for name, uf, ub in (("l1u", kw["d1f_U"], kw["d1b_U"]),
                             ("l2u", kw["d2f_U"], kw["d2b_U"])):
            o, _ = W_COLS[name]
            wpack[0:H, o + gi * 128: o + gi * 128 + H] = sgn * uf[:, gs]
            wpack[H:2 * H, o + gi * 128 + H: o + (gi + 1) * 128] = sgn * ub[:, gs]
        o, _ = W_COLS["l2a"]
        wpack[0:H, o + gi * 128: o + gi * 128 + H] = sgn * kw["d2f_W"][0:H, gs]
        wpack[H:2 * H, o + gi * 128 + H: o + (gi + 1) * 128] = \
            sgn * kw["d2b_W"][H:2 * H, gs]
        o, _ = W_COLS["l2b"]
        wpack[0:H, o + gi * 128 + H: o + (gi + 1) * 128] = \
            sgn * kw["d2b_W"][0:H, gs]
        wpack[H:2 * H, o + gi * 128: o + gi * 128 + H] = \
            sgn * kw["d2f_W"][H:2 * H, gs]

    o, _ = W_COLS["ident"]
    wpack[:, o:o + 128] = np.eye(2 * H, dtype=f32)
    o, _ = W_COLS["dense_w"]
    wpack[:, o:o + 128] = np.asarray(kw["dense_W"], f32)
    o, _ = W_COLS["out_w"]
    wpack[:, o:o + P] = np.asarray(kw["out_W"], f32)

    # ---- packed fp32 vectors [128, NV] ----
    vpack = np.zeros((128, NV), f32)
    gh, gz, gr = _gs("h"), _gs("z"), _gs("r")
    vpack[:, V_COLS["br1h"]] = np.concatenate([kw["d1f_br"][gh], kw["d1b_br"][gh]])
    vpack[:, V_COLS["br2h"]] = np.concatenate([kw["d2f_br"][gh], kw["d2b_br"][gh]])
    vpack[:, V_COLS["bi2h"]] = np.concatenate([kw["d2f_bi"][gh], kw["d2b_bi"][gh]])
    vpack[:, V_COLS["bz2n"]] = -np.concatenate(
        [kw["d2f_bi"][gz] + kw["d2f_br"][gz], kw["d2b_bi"][gz] + kw["d2b_br"][gz]])
    vpack[:, V_COLS["br2"]] = np.concatenate(
        [kw["d2f_bi"][gr] + kw["d2f_br"][gr], kw["d2b_bi"][gr] + kw["d2b_br"][gr]])
    vpack[:, V_COLS["dense_b"]] = np.asarray(kw["dense_b"], f32)
    vpack[0:P, V_COLS["out_b"]] = np.asarray(kw["out_b"], f32)
    split_l2 = bool(np.any(vpack[:, V_COLS["bz2n"]]) or
                    np.any(vpack[:, V_COLS["br2"]]))

    base = {
        "wpack": np.ascontiguousarray(wpack.astype(NPDT)),
        "vpack": np.ascontiguousarray(vpack),
    }

    in_maps = []
    for cid in range(NCORES):
        bs = slice(cid * BC, (cid + 1) * BC)
        xc = x[bs]  # (BC, T, F)
        xin19 = np.empty((NF, T, BC), f32)
        xin19[0:F] = xc.transpose(2, 1, 0)
        xin19[F:2 * F] = xc[:, ::-1, :].transpose(2, 1, 0)
        xin19[2 * F] = 1.0
        # 3-row-group layout: step s -> rows 32*(s%3), col chunk s//3
        xin32 = np.zeros((128, TC * BC), f32)
        for grp in range(3):
            xin32[32 * grp: 32 * grp + NF] = \
                xin19[:, grp::3, :].reshape(NF, TC * BC)
        xin32 = xin32.astype(NPDT)
        h0 = np.concatenate([np.asarray(kw["h0_fwd"], f32)[bs].T,
                             np.asarray(kw["h0_bwd"], f32)[bs].T], axis=0)
        m = dict(base)
        half = (TC // 2) * BC
        m["xin0"] = np.ascontiguousarray(xin32[:, :BC])
        m["xin1"] = np.ascontiguousarray(xin32[:, BC:half])
        m["xin2"] = np.ascontiguousarray(xin32[:, half:])
        m["h0"] = np.ascontiguousarray(h0.astype(NPDT))
        in_maps.append(m)
    return in_maps, split_l2


_CACHE = {}


def kernel(**inputs) -> np.ndarray:
    in_maps, split_l2 = prepare_maps(inputs)
    key = ("mod", split_l2)
    if key not in _CACHE:
        _CACHE[key] = build_module(split_l2)
    nc = _CACHE[key]
    res = run_bass_kernel_spmd(nc, in_maps, core_ids=list(range(NCORES)))
    outs = [r["out"] for r in res.results]  # each (P, BC)
    full = np.concatenate([o.T for o in outs], axis=0)  # (B, P)
    return np.ascontiguousarray(full.astype(np.float32))
